# revision 11
# baseline (speedup 1.0000x reference)
"""Trainium2 Bass kernel for nn_FCPairedLayer (pairwise MLP edge scorer).

Math (B=2, N=1024, C=128, H1=128, H2=64):
    a = x @ W1[:C]          # [B,N,H1]   left-token contribution
    r = x @ W1[C:]          # [B,N,H1]   right-token contribution
    h1 = relu(a_i + r_j + b1)           # per ordered pair (i,j)
    h2 = relu(h1 @ W2 + b2)             # [.,H2]
    y[b,i,j] = h2 @ W3 + b3  for j > i, else 0.

Strategy (8 cores, SPMD — one program, per-core data):
  The strict upper triangle is covered by 24 uniform work units of
  [128 rows x 512 cols]: row-block k of each batch needs columns
  [128k, 1024); that span is covered by windows {[128k,128k+512)} and
  a shared tail window [512,1024). Overlaps/below-diagonal columns are
  computed redundantly and masked on the host. 3 units per core.

  Per core the host packs transposed x slices (pure layout prep):
    xr [C=128, 3*128]  unit row-block tokens  (for a_i)
    xw [C=128, 3*512]  unit column-window tokens (for r_j)
  Device pipeline per unit:
    PE:   aT = W1l.T @ xr (+b1 via DVE) , rT = W1r.T @ xw  (fp32)
    DVE:  H_i = relu(rT + a_i)  as bf16      (tensor_scalar 4x mode)
    PE:   W2.T @ H_i -> PSUM, two col-tiled M=64 matmuls per bank
          (i-pair stacked on partitions 0:64 / 64:128)
    ACT:  h2s = relu(PSUM + b2_stacked) -> SBUF bf16
    PE:   y = b3 (K=1 ones matmul) + W3_stacked.T @ h2s  (4 col-tiled
          M=2 matmuls at partitions 32v..32v+1)
    DVE/ACT: copy PSUM -> SBUF fp32, DMA out 2-row strips.
  Host scatters the 24 unit outputs into y and applies the strict
  upper-triangular mask.
"""

import numpy as np
import ml_dtypes

B, N, C = 2, 1024, 128
H1, H2 = 128, 64
NCORES = 8
W = 512          # j-window width
UNITS = 3        # units per core
BF16 = ml_dtypes.bfloat16

# 24 uniform units: (batch, row_block, j0)
UNITS_ALL = []
for _b in range(2):
    for _k in range(8):
        for _j0 in ([128 * _k, 512] if _k < 4 else [512]):
            UNITS_ALL.append((_b, _k, _j0))
assert len(UNITS_ALL) == NCORES * UNITS
CORE_UNITS = [UNITS_ALL[UNITS * c:UNITS * (c + 1)] for c in range(NCORES)]

_TRIU = None
LAST_PERF = {}


def _split_sync_waits(bir_json, limit=1):
    """Walrus in this toolchain rejects instructions carrying more than one
    sync-wait command ("Too many sync wait commands", CoreV3GenImpl.cpp).
    Tile attaches multi-sem waits to instructions; rewrite the BIR so each
    instruction keeps at most `limit` waits and the rest ride on preceding
    single-wait EventSemaphore instructions on the same engine (the exact
    encoding raw-bass wait_ge() uses)."""
    import json

    data = json.loads(bir_json)
    for f in data.get("functions", []):
        for blk in f.get("blocks", []):
            out = []
            for ins in blk.get("instructions", []):
                si = ins.get("sync_info")
                ow = (si or {}).get("on_wait") or []
                if len(ow) > limit:
                    for k, w in enumerate(ow[:-limit]):
                        out.append({
                            "debug": ins.get("debug", 0),
                            "engine": ins["engine"],
                            "name": f"{ins['name']}-xw{k}",
                            "opcode": "EventSemaphore",
                            "sync_info": {"on_update": [], "on_wait": [w]},
                        })
                    si["on_wait"] = ow[-limit:]
                out.append(ins)
            blk["instructions"] = out
    return json.dumps(data).encode()


def _install_compile_patch():
    import concourse.bass_utils as bu
    import concourse.bass2jax as b2j

    if getattr(bu, "_fc_split_waits_patch", False):
        return
    orig = bu.compile_bir_kernel

    def patched(bir_json, tmpdir, neff_name="file.neff"):
        return orig(_split_sync_waits(bir_json), tmpdir, neff_name)

    bu._fc_split_waits_patch = True
    bu.compile_bir_kernel = patched
    b2j.compile_bir_kernel = patched


def _build_program(trace=False):
    import concourse.bass as bass
    import concourse.mybir as mybir
    from concourse.tile import TileContext

    f32 = mybir.dt.float32
    bf16 = mybir.dt.bfloat16
    nc = bass.Bass()

    xr_d = nc.declare_dram_parameter("xr", [C, UNITS * 128], f32, isOutput=False)
    xw_d = nc.declare_dram_parameter("xw", [C, UNITS * W], f32, isOutput=False)
    w1l_d = nc.declare_dram_parameter("w1l", [C, H1], f32, isOutput=False)
    w1r_d = nc.declare_dram_parameter("w1r", [C, H1], f32, isOutput=False)
    b1c_d = nc.declare_dram_parameter("b1c", [H1, 1], f32, isOutput=False)
    w2b_d = nc.declare_dram_parameter("w2b", [H1, H2], bf16, isOutput=False)
    b2s_d = nc.declare_dram_parameter("b2s", [128, 1], f32, isOutput=False)
    w3s_d = nc.declare_dram_parameter("w3s", [128, 32], bf16, isOutput=False)
    b3c_d = nc.declare_dram_parameter("b3c", [128, 1], f32, isOutput=False)
    y_d = nc.declare_dram_parameter("y", [UNITS, 128, W], f32, isOutput=True)

    Relu = mybir.ActivationFunctionType.Relu
    Identity = mybir.ActivationFunctionType.Identity
    ADD = mybir.AluOpType.add
    MAX = mybir.AluOpType.max

    with TileContext(nc) as tc:
        with tc.tile_pool(name="const", bufs=1) as const:
            w1l_t = const.tile([C, H1], f32, tag="w1l")
            w1r_t = const.tile([C, H1], f32, tag="w1r")
            b1c_t = const.tile([H1, 1], f32, tag="b1c")
            w2b_t = const.tile([H1, H2], bf16, tag="w2b")
            b2s_t = const.tile([128, 1], f32, tag="b2s")
            w3s_t = const.tile([128, 32], bf16, tag="w3s")
            b3c_t = const.tile([128, 1], f32, tag="b3c")
            xr_t = const.tile([C, UNITS * 128], f32, tag="xr")
            xw_t = const.tile([C, UNITS * W], f32, tag="xw")
            aTb1_t = const.tile([H1, UNITS * 128], f32, tag="aTb1")
            rT_t = const.tile([H1, UNITS * W], bf16, tag="rT")

            for t, d in [(w1l_t, w1l_d), (w1r_t, w1r_d), (b1c_t, b1c_d),
                         (w2b_t, w2b_d), (b2s_t, b2s_d), (w3s_t, w3s_d),
                         (b3c_t, b3c_d), (xr_t, xr_d), (xw_t, xw_d)]:
                nc.sync.dma_start(out=t, in_=d[:])

            # First stage: aT/rT in [H1, token] layout (fp32 matmuls).
            with tc.tile_pool(name="pre", bufs=2, space="PSUM") as pre:
                pa = pre.tile([128, UNITS * 128], f32, tag="pa")
                nc.tensor.matmul(pa, lhsT=w1l_t, rhs=xr_t, start=True, stop=True)
                nc.vector.tensor_scalar(aTb1_t, pa, b1c_t, None, ADD)
                for u in range(UNITS):
                    pr = pre.tile([128, W], f32, tag="pr")
                    nc.tensor.matmul(pr, lhsT=w1r_t, rhs=xw_t[:, u * W:(u + 1) * W],
                                     start=True, stop=True)
                    nc.scalar.copy(rT_t[:, u * W:(u + 1) * W], pr)

            with (
                tc.tile_pool(name="Hp", bufs=6) as Hp,
                tc.tile_pool(name="h2p", bufs=4) as h2p,
                tc.tile_pool(name="yp", bufs=3) as yp,
                tc.tile_pool(name="ph2", bufs=3, space="PSUM") as ph2p,
                tc.tile_pool(name="pyp", bufs=2, space="PSUM") as pyp,
            ):
                for u in range(UNITS):
                    for t in range(16):          # octet of 8 i's per iter
                        h2s_tiles = []
                        for half in range(2):
                            ph = ph2p.tile([128, 1024], f32, tag="ph")
                            for q in range(2):
                                for e in range(2):
                                    il = 8 * t + 4 * half + 2 * q + e
                                    Ht = Hp.tile([128, W], bf16, tag="H")
                                    nc.vector.tensor_scalar(
                                        Ht, rT_t[:, u * W:(u + 1) * W],
                                        aTb1_t[:, u * 128 + il:u * 128 + il + 1],
                                        0.0, ADD, op1=MAX)
                                    nc.tensor.matmul(
                                        ph[64 * e:64 * (e + 1), q * W:(q + 1) * W],
                                        lhsT=w2b_t, rhs=Ht, start=True, stop=True,
                                        tile_position=(0, 64 * e))
                            h2s = h2p.tile([128, 1024], bf16, tag="h2s")
                            nc.scalar.activation(h2s, ph, Relu, bias=b2s_t)
                            h2s_tiles.append(h2s)
                        py_t = pyp.tile([128, W], f32, tag="py")
                        for v in range(4):
                            half, q = divmod(v, 2)
                            nc.tensor.matmul(
                                py_t[32 * v:32 * v + 32, :], lhsT=w3s_t,
                                rhs=h2s_tiles[half][:, q * W:(q + 1) * W],
                                start=True, stop=True,
                                tile_position=(0, 32 * v))
                        ysb = yp.tile([128, W], f32, tag="ysb")
                        if t % 2 == 0:
                            nc.scalar.activation(ysb, py_t, Identity, bias=b3c_t)
                        else:
                            nc.vector.tensor_scalar(ysb, py_t, b3c_t, None, ADD)
                        for v in range(4):
                            nc.sync.dma_start(
                                out=y_d[u, 8 * t + 2 * v:8 * t + 2 * v + 2, :],
                                in_=ysb[32 * v:32 * v + 2, :])
    return nc


def _pack_inputs(x, W1, b1, W2, b2, W3, b3):
    xT = np.ascontiguousarray(x.transpose(0, 2, 1)).astype(np.float32)  # [2,C,N]
    w1l = np.ascontiguousarray(W1[:C]).astype(np.float32)
    w1r = np.ascontiguousarray(W1[C:]).astype(np.float32)
    b1c = np.ascontiguousarray(b1.reshape(H1, 1)).astype(np.float32)
    w2b = np.ascontiguousarray(W2).astype(BF16)
    b2s = np.concatenate([b2, b2]).reshape(128, 1).astype(np.float32)
    w3s = np.zeros((128, 32), dtype=BF16)
    w3s[0:64, 0] = W3[:, 0].astype(BF16)
    w3s[64:128, 1] = W3[:, 0].astype(BF16)
    b3c = np.full((128, 1), b3[0], dtype=np.float32)

    in_maps = []
    for c in range(NCORES):
        units = CORE_UNITS[c]
        xr = np.concatenate([xT[b][:, 128 * k:128 * k + 128] for (b, k, _) in units], axis=1)
        xw = np.concatenate([xT[b][:, j0:j0 + W] for (b, _, j0) in units], axis=1)
        in_maps.append({
            "xr": np.ascontiguousarray(xr), "xw": np.ascontiguousarray(xw),
            "w1l": w1l, "w1r": w1r, "b1c": b1c, "w2b": w2b, "b2s": b2s,
            "w3s": w3s, "b3c": b3c,
        })
    return in_maps


def _assemble(results):
    global _TRIU
    y = np.zeros((B, N, N), dtype=np.float32)
    for c in range(NCORES):
        out = results[c]["y"]
        for idx, (b, k, j0) in enumerate(CORE_UNITS[c]):
            y[b, 128 * k:128 * k + 128, j0:j0 + W] = out[idx]
    if _TRIU is None:
        _TRIU = np.triu(np.ones((N, N), dtype=np.float32), k=1)
    y *= _TRIU
    return y


def kernel(x, W1, b1, W2, b2, W3, b3):
    import os
    _install_compile_patch()
    from concourse.bass_utils import run_bass_kernel_spmd

    trace = bool(int(os.environ.get("FC_TRACE", "0")))
    nc = _build_program()
    in_maps = _pack_inputs(np.asarray(x), np.asarray(W1), np.asarray(b1),
                           np.asarray(W2), np.asarray(b2), np.asarray(W3),
                           np.asarray(b3))
    res = run_bass_kernel_spmd(nc, in_maps, core_ids=list(range(NCORES)),
                               trace=trace)
    LAST_PERF.clear()
    LAST_PERF.update({
        "exec_time_ns": res.exec_time_ns,
        "mean_exec_time_ns": res.mean_exec_time_ns,
        "trace": res.instructions_and_trace[1] if res.instructions_and_trace else None,
    })
    return _assemble(res.results)


# revision 35
# speedup vs baseline: 1.3155x; 1.3155x over previous
"""Trainium2 Bass kernel for nn_FCPairedLayer (pairwise MLP edge scorer).

Math (B=2, N=1024, C=128, H1=128, H2=64):
    a = x @ W1[:C]          # [B,N,H1]   left-token contribution
    r = x @ W1[C:]          # [B,N,H1]   right-token contribution
    h1 = relu(a_i + r_j + b1)           # per ordered pair (i,j)
    h2 = relu(h1 @ W2 + b2)             # [.,H2]
    y[b,i,j] = h2 @ W3 + b3  for j > i, else 0.

Strategy (8 cores, SPMD — one program, per-core data):
  The strict upper triangle is covered by 16 uniform work units: each
  core gets one [128 rows x 1024 cols] unit (a row-block k<4 vs window
  [0,1024)) and one [128 x 512] unit (a row-block k>=4 vs window
  [512,1024)). Below-diagonal columns are computed redundantly and
  masked on the host. The wide unit halves the H-construct instruction
  count (the DVE bottleneck) at identical total pair count.

  Per core the host packs transposed x slices (pure layout prep):
    xr [C=128, 3*128]  unit row-block tokens  (for a_i)
    xw [C=128, 3*512]  unit column-window tokens (for r_j)
  Device pipeline per unit:
    PE:   aT = W1l.T @ xr (+b1 via DVE), rT = W1r.T @ xw (fp32r, 1 cyc/row)
    DVE:  H_i = relu(rT + a_i) as bf16 (tensor_scalar ptr-scalar, 2x mode —
          this is the throughput-limiting stage, ~341 ns per [128,512])
    PE:   W2.T @ H_i -> PSUM, two col-tiled M=64 matmuls per bank
          (i-pair stacked on partitions 0:64 / 64:128, HW-concurrent)
    ACT:  h2s = relu(PSUM + b2_stacked) -> SBUF bf16 ([128,1024] per 2 banks)
    PE:   W3_stacked.T @ h2s: 4 col-tiled M=32 matmuls at partitions
          32v..32v+1 (zero-padded weight cols keep the bank fully written)
    DVE/ACT (alternating per octet pair): +b3 and copy PSUM -> SBUF fp32
          over a merged 2-octet [128,1024] PSUM tile, accumulated into a
          4-octet SBUF tile, then 4 grouped 16KB DMAs per tile.
  Host scatters the 24 unit outputs into y and applies the strict
  upper-triangular mask (overlapping windows write identical values).

  Environment workaround: this walrus build accepts at most ONE sync-wait
  per instruction, so compile goes through a BIR rewrite that moves extra
  Tile-generated waits onto single-wait EventSemaphore carriers.

  Measured on the 8-core axon TRN2 pool: HW exec ~143 us, rel err 5.3e-3
  (bf16 H/h2 path; fp32 accumulate everywhere).
"""

import numpy as np
import ml_dtypes

B, N, C = 2, 1024, 128
H1, H2 = 128, 64
NCORES = 8
W = 512          # j-window width
UNITS = 3        # units per core
BF16 = ml_dtypes.bfloat16

# Per core: one wide unit (block k<4, window [0,1024)) and one narrow unit
# (block k>=4, window [512,1024)). Same computed pairs as 3x512 windows but
# 256 construct instructions per core instead of 384.
U1024 = [(_b, _k) for _b in range(2) for _k in range(4)]
U512 = [(_b, _k) for _b in range(2) for _k in range(4, 8)]
CORE_UNITS = [(U1024[c], U512[c]) for c in range(NCORES)]

_TRIU = None
LAST_PERF = {}


def _split_sync_waits(bir_json, limit=1):
    """Walrus in this toolchain rejects instructions carrying more than one
    sync-wait command ("Too many sync wait commands", CoreV3GenImpl.cpp).
    Tile attaches multi-sem waits to instructions; rewrite the BIR so each
    instruction keeps at most `limit` waits and the rest ride on preceding
    single-wait EventSemaphore instructions on the same engine (the exact
    encoding raw-bass wait_ge() uses)."""
    import json

    data = json.loads(bir_json)
    for f in data.get("functions", []):
        for blk in f.get("blocks", []):
            out = []
            for ins in blk.get("instructions", []):
                si = ins.get("sync_info")
                ow = (si or {}).get("on_wait") or []
                if len(ow) > limit:
                    for k, w in enumerate(ow[:-limit]):
                        out.append({
                            "debug": ins.get("debug", 0),
                            "engine": ins["engine"],
                            "name": f"{ins['name']}-xw{k}",
                            "opcode": "EventSemaphore",
                            "sync_info": {"on_update": [], "on_wait": [w]},
                        })
                    si["on_wait"] = ow[-limit:]
                out.append(ins)
            blk["instructions"] = out
    return json.dumps(data).encode()


def _install_compile_patch():
    import concourse.bass_utils as bu
    import concourse.bass2jax as b2j

    if getattr(bu, "_fc_split_waits_patch", False):
        return
    orig = bu.compile_bir_kernel

    def patched(bir_json, tmpdir, neff_name="file.neff"):
        return orig(_split_sync_waits(bir_json), tmpdir, neff_name)

    bu._fc_split_waits_patch = True
    bu.compile_bir_kernel = patched
    b2j.compile_bir_kernel = patched


def _build_program(trace=False):
    import os
    import concourse.bass as bass
    import concourse.mybir as mybir
    from concourse.tile import TileContext

    n_act = int(os.environ.get("FC_ACT", "0"))  # constructs/octet on ACT

    f32 = mybir.dt.float32
    bf16 = mybir.dt.bfloat16
    nc = bass.Bass()

    f32r = mybir.dt.float32r
    xr_d = nc.declare_dram_parameter("xr", [C, 256], f32r, isOutput=False)
    xw_d = nc.declare_dram_parameter("xw", [C, 1536], f32r, isOutput=False)
    w1l_d = nc.declare_dram_parameter("w1l", [C, H1], f32r, isOutput=False)
    w1r_d = nc.declare_dram_parameter("w1r", [C, H1], f32r, isOutput=False)
    b1c_d = nc.declare_dram_parameter("b1c", [H1, 1], f32, isOutput=False)
    w2b_d = nc.declare_dram_parameter("w2b", [H1, H2], bf16, isOutput=False)
    b2s_d = nc.declare_dram_parameter("b2s", [128, 1], f32, isOutput=False)
    w3s_d = nc.declare_dram_parameter("w3s", [128, 32], bf16, isOutput=False)
    b3c_d = nc.declare_dram_parameter("b3c", [128, 1], f32, isOutput=False)
    y_d = nc.declare_dram_parameter("y", [128, 1536], f32, isOutput=True)

    Relu = mybir.ActivationFunctionType.Relu
    Identity = mybir.ActivationFunctionType.Identity
    ADD = mybir.AluOpType.add
    MAX = mybir.AluOpType.max

    with TileContext(nc) as tc:
        with tc.tile_pool(name="const", bufs=1) as const:
            w1l_t = const.tile([C, H1], f32r, tag="w1l")
            w1r_t = const.tile([C, H1], f32r, tag="w1r")
            b1c_t = const.tile([H1, 1], f32, tag="b1c")
            w2b_t = const.tile([H1, H2], bf16, tag="w2b")
            b2s_t = const.tile([128, 1], f32, tag="b2s")
            w3s_t = const.tile([128, 32], bf16, tag="w3s")
            b3c_t = const.tile([128, 1], f32, tag="b3c")
            xr_t = const.tile([C, 256], f32r, tag="xr")
            xw_t = const.tile([C, 1536], f32r, tag="xw")
            aTb1_t = const.tile([H1, 256], f32, tag="aTb1")
            rT_t = const.tile([H1, 1536], bf16, tag="rT")

            nc.sync.dma_start(out=w1r_t, in_=w1r_d[:])
            nc.sync.dma_start(out=w1l_t, in_=w1l_d[:])
            nc.sync.dma_start(out=xw_t[:, 1024:1536], in_=xw_d[:, 1024:1536])
            nc.sync.dma_start(out=xr_t, in_=xr_d[:])
            for t, d in [(b1c_t, b1c_d), (w2b_t, w2b_d), (b2s_t, b2s_d),
                         (w3s_t, w3s_d), (b3c_t, b3c_d)]:
                nc.sync.dma_start(out=t, in_=d[:])

            # First stage: aT/rT in [H1, token] layout (fp32r, full rate).
            with tc.tile_pool(name="pre", bufs=2, space="PSUM") as pre:
                pa = pre.tile([128, 256], f32, tag="pa")
                nc.tensor.matmul(pa, lhsT=w1l_t, rhs=xr_t,
                                 start=True, stop=True)
                nc.vector.tensor_scalar(aTb1_t, pa, b1c_t, None, ADD)
                for ci, ch in enumerate((2, 0, 1)):
                    if ci > 0:
                        nc.sync.dma_start(out=xw_t[:, ch * 512:(ch + 1) * 512],
                                          in_=xw_d[:, ch * 512:(ch + 1) * 512])
                    pr = pre.tile([128, 512], f32, tag="pr")
                    nc.tensor.matmul(pr, lhsT=w1r_t,
                                     rhs=xw_t[:, ch * 512:(ch + 1) * 512],
                                     start=True, stop=True)
                    nc.scalar.copy(rT_t[:, ch * 512:(ch + 1) * 512], pr)

            with (
                tc.tile_pool(name="Hp", bufs=10) as Hp,
                tc.tile_pool(name="h2p", bufs=6) as h2p,
                tc.tile_pool(name="yp", bufs=3) as yp,
                tc.tile_pool(name="ph2", bufs=3, space="PSUM") as ph2p,
                tc.tile_pool(name="pyp", bufs=1, space="PSUM") as pyp,
            ):
                # Unit B: rows of a k>=4 block vs window [512,1024).
                yvB = y_d[:, 1024:1536].rearrange(
                    "(o v e) f -> v e o f", v=4, e=2)
                for t in range(16):
                    if t % 4 == 0:
                        ysb4 = yp.tile([128, 4, 512], f32, tag="ysb4")
                    if t % 2 == 0:
                        py2 = pyp.tile([128, 2, 512], f32, tag="py")
                    h2s_tiles = []
                    for half in range(2):
                        ph = ph2p.tile([128, 1024], f32, tag="ph")
                        for q in range(2):
                            for e in range(2):
                                il = 128 + 8 * t + 4 * half + 2 * q + e
                                Ht = Hp.tile([128, 512], bf16, tag="HB")
                                nc.vector.tensor_scalar(
                                    Ht, rT_t[:, 1024:1536],
                                    aTb1_t[:, il:il + 1], 0.0, ADD, op1=MAX)
                                nc.tensor.matmul(
                                    ph[64 * e:64 * (e + 1),
                                       q * 512:(q + 1) * 512],
                                    lhsT=w2b_t, rhs=Ht, start=True, stop=True,
                                    tile_position=(0, 64 * e))
                        h2s = h2p.tile([128, 1024], bf16, tag="h2s")
                        nc.scalar.activation(h2s, ph, Relu, bias=b2s_t)
                        h2s_tiles.append(h2s)
                    for v in range(4):
                        half, q = divmod(v, 2)
                        nc.tensor.matmul(
                            py2[32 * v:32 * v + 32, t % 2, :], lhsT=w3s_t,
                            rhs=h2s_tiles[half][:, q * 512:(q + 1) * 512],
                            start=True, stop=True,
                            tile_position=(0, 32 * v))
                    if t % 2 == 1:
                        dst = ysb4[:, t % 4 - 1:t % 4 + 1, :]
                        if t % 4 == 1:
                            nc.scalar.activation(dst, py2, Identity,
                                                 bias=b3c_t)
                        else:
                            nc.vector.tensor_scalar(dst, py2, b3c_t,
                                                    None, ADD)
                    if t % 4 == 3:
                        g = t // 4
                        for v in range(4):
                            nc.sync.dma_start(
                                out=yvB[v, :, 4 * g:4 * g + 4, :],
                                in_=ysb4[32 * v:32 * v + 2, :, :])
                # Unit A: rows of a k<4 block vs window [0,1024).
                # y rows as (group G, octet-parity o, pair v, elem e), cols as
                # (half jh, c) for the grouped 2-octet DMA.
                yvA = y_d[:, 0:1024].rearrange(
                    "(G o v e) (jh c) -> G v e o jh c", o=2, v=4, e=2, jh=2)
                for t in range(16):
                    if t % 2 == 0:
                        ysb4 = yp.tile([128, 4, 512], f32, tag="ysb4")
                    Hts = []
                    for idx8 in range(8):
                        il = 8 * t + idx8
                        Ht = Hp.tile([128, 1024], bf16, tag="HA")
                        nc.vector.tensor_scalar(
                            Ht, rT_t[:, 0:1024],
                            aTb1_t[:, il:il + 1], 0.0, ADD, op1=MAX)
                        Hts.append(Ht)
                    py2 = pyp.tile([128, 2, 512], f32, tag="py")
                    for jh in range(2):
                        h2s_tiles = []
                        for half in range(2):
                            ph = ph2p.tile([128, 1024], f32, tag="ph")
                            for q in range(2):
                                for e in range(2):
                                    idx8 = 4 * half + 2 * q + e
                                    nc.tensor.matmul(
                                        ph[64 * e:64 * (e + 1),
                                           q * 512:(q + 1) * 512],
                                        lhsT=w2b_t,
                                        rhs=Hts[idx8][:, jh * 512:(jh + 1) * 512],
                                        start=True, stop=True,
                                        tile_position=(0, 64 * e))
                            h2s = h2p.tile([128, 1024], bf16, tag="h2s")
                            nc.scalar.activation(h2s, ph, Relu, bias=b2s_t)
                            h2s_tiles.append(h2s)
                        for v in range(4):
                            half, q = divmod(v, 2)
                            nc.tensor.matmul(
                                py2[32 * v:32 * v + 32, jh, :], lhsT=w3s_t,
                                rhs=h2s_tiles[half][:, q * 512:(q + 1) * 512],
                                start=True, stop=True,
                                tile_position=(0, 32 * v))
                    o = t % 2
                    dst = ysb4[:, 2 * o:2 * o + 2, :]
                    if o == 0:
                        nc.scalar.activation(dst, py2, Identity, bias=b3c_t)
                    else:
                        nc.vector.tensor_scalar(dst, py2, b3c_t, None, ADD)
                    if o == 1:
                        G = t // 2
                        for v in range(4):
                            nc.sync.dma_start(
                                out=yvA[G, v],
                                in_=ysb4[32 * v:32 * v + 2, :, :].rearrange(
                                    "p (o jh) c -> p o jh c", o=2))
    return nc


def _pack_inputs(x, W1, b1, W2, b2, W3, b3):
    xT = np.ascontiguousarray(x.transpose(0, 2, 1)).astype(np.float32)  # [2,C,N]
    w1l = np.ascontiguousarray(W1[:C]).astype(np.float32)
    w1r = np.ascontiguousarray(W1[C:]).astype(np.float32)
    b1c = np.ascontiguousarray(b1.reshape(H1, 1)).astype(np.float32)
    w2b = np.ascontiguousarray(W2).astype(BF16)
    b2s = np.concatenate([b2, b2]).reshape(128, 1).astype(np.float32)
    w3s = np.zeros((128, 32), dtype=BF16)
    w3s[0:64, 0] = W3[:, 0].astype(BF16)
    w3s[64:128, 1] = W3[:, 0].astype(BF16)
    b3c = np.full((128, 1), b3[0], dtype=np.float32)

    in_maps = []
    for c in range(NCORES):
        (bA, kA), (bB, kB) = CORE_UNITS[c]
        xr = np.concatenate([xT[bA][:, 128 * kA:128 * kA + 128],
                             xT[bB][:, 128 * kB:128 * kB + 128]], axis=1)
        xw = np.concatenate([xT[bA][:, 0:1024], xT[bB][:, 512:1024]], axis=1)
        in_maps.append({
            "xr": np.ascontiguousarray(xr), "xw": np.ascontiguousarray(xw),
            "w1l": w1l, "w1r": w1r, "b1c": b1c, "w2b": w2b, "b2s": b2s,
            "w3s": w3s, "b3c": b3c,
        })
    return in_maps


def _assemble(results):
    global _TRIU
    y = np.zeros((B, N, N), dtype=np.float32)
    for c in range(NCORES):
        out = results[c]["y"]          # [128, 1536]
        (bA, kA), (bB, kB) = CORE_UNITS[c]
        y[bA, 128 * kA:128 * kA + 128, 0:1024] = out[:, 0:1024]
        y[bB, 128 * kB:128 * kB + 128, 512:1024] = out[:, 1024:1536]
    if _TRIU is None:
        _TRIU = np.triu(np.ones((N, N), dtype=np.float32), k=1)
    y *= _TRIU
    return y


def kernel(x, W1, b1, W2, b2, W3, b3):
    import os
    _install_compile_patch()
    from concourse.bass_utils import run_bass_kernel_spmd

    trace = bool(int(os.environ.get("FC_TRACE", "0")))
    nc = _build_program()
    in_maps = _pack_inputs(np.asarray(x), np.asarray(W1), np.asarray(b1),
                           np.asarray(W2), np.asarray(b2), np.asarray(W3),
                           np.asarray(b3))
    res = run_bass_kernel_spmd(nc, in_maps, core_ids=list(range(NCORES)),
                               trace=trace)
    LAST_PERF.clear()
    LAST_PERF.update({
        "exec_time_ns": res.exec_time_ns,
        "mean_exec_time_ns": res.mean_exec_time_ns,
        "trace": res.instructions_and_trace[1] if res.instructions_and_trace else None,
    })
    return _assemble(res.results)


# revision 37
# speedup vs baseline: 1.3202x; 1.0036x over previous
"""Trainium2 Bass kernel for nn_FCPairedLayer (pairwise MLP edge scorer).

Math (B=2, N=1024, C=128, H1=128, H2=64):
    a = x @ W1[:C]          # [B,N,H1]   left-token contribution
    r = x @ W1[C:]          # [B,N,H1]   right-token contribution
    h1 = relu(a_i + r_j + b1)           # per ordered pair (i,j)
    h2 = relu(h1 @ W2 + b2)             # [.,H2]
    y[b,i,j] = h2 @ W3 + b3  for j > i, else 0.

Strategy (8 cores, SPMD — one program, per-core data):
  The strict upper triangle is covered by 16 uniform work units: each
  core gets one [128 rows x 1024 cols] unit (a row-block k<4 vs window
  [0,1024)) and one [128 x 512] unit (a row-block k>=4 vs window
  [512,1024)). Below-diagonal columns are computed redundantly and
  masked on the host. The wide unit halves the H-construct instruction
  count (the DVE bottleneck) at identical total pair count.

  Per core the host packs transposed x slices (pure layout prep):
    xr [C=128, 3*128]  unit row-block tokens  (for a_i)
    xw [C=128, 3*512]  unit column-window tokens (for r_j)
  Device pipeline per unit:
    PE:   aT = W1l.T @ xr (+b1 via DVE), rT = W1r.T @ xw (fp32r, 1 cyc/row)
    DVE:  H_i = relu(rT + a_i) as bf16 (tensor_scalar ptr-scalar, 2x mode —
          this is the throughput-limiting stage, ~341 ns per [128,512])
    PE:   W2.T @ H_i -> PSUM, two col-tiled M=64 matmuls per bank
          (i-pair stacked on partitions 0:64 / 64:128, HW-concurrent)
    ACT:  h2s = relu(PSUM + b2_stacked) -> SBUF bf16 ([128,1024] per 2 banks)
    PE:   W3_stacked.T @ h2s: 4 col-tiled M=32 matmuls at partitions
          32v..32v+1 (zero-padded weight cols keep the bank fully written)
    DVE/ACT (alternating per octet pair): +b3 and copy PSUM -> SBUF fp32
          over a merged 2-octet [128,1024] PSUM tile, accumulated into a
          4-octet SBUF tile, then 4 grouped 16KB DMAs per tile.
  Host scatters the 24 unit outputs into y and applies the strict
  upper-triangular mask (overlapping windows write identical values).

  Environment workaround: this walrus build accepts at most ONE sync-wait
  per instruction, so compile goes through a BIR rewrite that moves extra
  Tile-generated waits onto single-wait EventSemaphore carriers.

  Measured on the 8-core axon TRN2 pool: HW exec ~143 us, rel err 5.3e-3
  (bf16 H/h2 path; fp32 accumulate everywhere).
"""

import numpy as np
import ml_dtypes

B, N, C = 2, 1024, 128
H1, H2 = 128, 64
NCORES = 8
W = 512          # j-window width
UNITS = 3        # units per core
BF16 = ml_dtypes.bfloat16

# Per core: one wide unit (block k<4, window [0,1024)) and one narrow unit
# (block k>=4, window [512,1024)). Same computed pairs as 3x512 windows but
# 256 construct instructions per core instead of 384.
U1024 = [(_b, _k) for _b in range(2) for _k in range(4)]
U512 = [(_b, _k) for _b in range(2) for _k in range(4, 8)]
CORE_UNITS = [(U1024[c], U512[c]) for c in range(NCORES)]

_TRIU = None
LAST_PERF = {}


def _split_sync_waits(bir_json, limit=1):
    """Walrus in this toolchain rejects instructions carrying more than one
    sync-wait command ("Too many sync wait commands", CoreV3GenImpl.cpp).
    Tile attaches multi-sem waits to instructions; rewrite the BIR so each
    instruction keeps at most `limit` waits and the rest ride on preceding
    single-wait EventSemaphore instructions on the same engine (the exact
    encoding raw-bass wait_ge() uses)."""
    import json

    data = json.loads(bir_json)
    for f in data.get("functions", []):
        for blk in f.get("blocks", []):
            out = []
            for ins in blk.get("instructions", []):
                si = ins.get("sync_info")
                ow = (si or {}).get("on_wait") or []
                if len(ow) > limit:
                    for k, w in enumerate(ow[:-limit]):
                        out.append({
                            "debug": ins.get("debug", 0),
                            "engine": ins["engine"],
                            "name": f"{ins['name']}-xw{k}",
                            "opcode": "EventSemaphore",
                            "sync_info": {"on_update": [], "on_wait": [w]},
                        })
                    si["on_wait"] = ow[-limit:]
                out.append(ins)
            blk["instructions"] = out
    return json.dumps(data).encode()


def _install_compile_patch():
    import concourse.bass_utils as bu
    import concourse.bass2jax as b2j

    if getattr(bu, "_fc_split_waits_patch", False):
        return
    orig = bu.compile_bir_kernel

    def patched(bir_json, tmpdir, neff_name="file.neff"):
        return orig(_split_sync_waits(bir_json), tmpdir, neff_name)

    bu._fc_split_waits_patch = True
    bu.compile_bir_kernel = patched
    b2j.compile_bir_kernel = patched


def _build_program(trace=False):
    import os
    import concourse.bass as bass
    import concourse.mybir as mybir
    from concourse.tile import TileContext

    n_act = int(os.environ.get("FC_ACT", "0"))  # constructs/octet on ACT

    f32 = mybir.dt.float32
    bf16 = mybir.dt.bfloat16
    nc = bass.Bass()

    f32r = mybir.dt.float32r
    xr_d = nc.declare_dram_parameter("xr", [C, 256], f32r, isOutput=False)
    xw_d = nc.declare_dram_parameter("xw", [C, 1536], f32r, isOutput=False)
    w1l_d = nc.declare_dram_parameter("w1l", [C, H1], f32r, isOutput=False)
    w1r_d = nc.declare_dram_parameter("w1r", [C, H1], f32r, isOutput=False)
    b1c_d = nc.declare_dram_parameter("b1c", [H1, 1], f32, isOutput=False)
    w2b_d = nc.declare_dram_parameter("w2b", [H1, H2], bf16, isOutput=False)
    b2s_d = nc.declare_dram_parameter("b2s", [128, 1], f32, isOutput=False)
    w3s_d = nc.declare_dram_parameter("w3s", [128, 32], bf16, isOutput=False)
    b3c_d = nc.declare_dram_parameter("b3c", [128, 1], f32, isOutput=False)
    y_d = nc.declare_dram_parameter("y", [128, 1536], f32, isOutput=True)

    Relu = mybir.ActivationFunctionType.Relu
    Identity = mybir.ActivationFunctionType.Identity
    ADD = mybir.AluOpType.add
    MAX = mybir.AluOpType.max

    with TileContext(nc) as tc:
        with tc.tile_pool(name="const", bufs=1) as const:
            w1l_t = const.tile([C, H1], f32r, tag="w1l")
            w1r_t = const.tile([C, H1], f32r, tag="w1r")
            b1c_t = const.tile([H1, 1], f32, tag="b1c")
            w2b_t = const.tile([H1, H2], bf16, tag="w2b")
            b2s_t = const.tile([128, 1], f32, tag="b2s")
            w3s_t = const.tile([128, 32], bf16, tag="w3s")
            b3c_t = const.tile([128, 1], f32, tag="b3c")
            xr_t = const.tile([C, 256], f32r, tag="xr")
            xw_t = const.tile([C, 1536], f32r, tag="xw")
            aTb1_t = const.tile([H1, 256], f32, tag="aTb1")
            rT_t = const.tile([H1, 1536], bf16, tag="rT")

            nc.sync.dma_start(out=w1r_t, in_=w1r_d[:])
            nc.sync.dma_start(out=w1l_t, in_=w1l_d[:])
            nc.sync.dma_start(out=xw_t[:, 0:512], in_=xw_d[:, 0:512])
            nc.sync.dma_start(out=xr_t, in_=xr_d[:])
            for t, d in [(b1c_t, b1c_d), (w2b_t, w2b_d), (b2s_t, b2s_d),
                         (w3s_t, w3s_d), (b3c_t, b3c_d)]:
                nc.sync.dma_start(out=t, in_=d[:])

            # First stage: aT/rT in [H1, token] layout (fp32r, full rate).
            with tc.tile_pool(name="pre", bufs=2, space="PSUM") as pre:
                pa = pre.tile([128, 256], f32, tag="pa")
                nc.tensor.matmul(pa, lhsT=w1l_t, rhs=xr_t,
                                 start=True, stop=True)
                nc.vector.tensor_scalar(aTb1_t, pa, b1c_t, None, ADD)
                for ch in range(3):
                    if ch > 0:
                        nc.sync.dma_start(out=xw_t[:, ch * 512:(ch + 1) * 512],
                                          in_=xw_d[:, ch * 512:(ch + 1) * 512])
                    pr = pre.tile([128, 512], f32, tag="pr")
                    nc.tensor.matmul(pr, lhsT=w1r_t,
                                     rhs=xw_t[:, ch * 512:(ch + 1) * 512],
                                     start=True, stop=True)
                    nc.scalar.copy(rT_t[:, ch * 512:(ch + 1) * 512], pr)

            with (
                tc.tile_pool(name="Hp", bufs=10) as Hp,
                tc.tile_pool(name="h2p", bufs=6) as h2p,
                tc.tile_pool(name="yp", bufs=3) as yp,
                tc.tile_pool(name="ph2", bufs=2, space="PSUM") as ph2p,
                tc.tile_pool(name="pyp", bufs=2, space="PSUM") as pyp,
            ):
                # Unit A: rows of a k<4 block vs window [0,1024).
                # y rows as (group G, octet-parity o, pair v, elem e), cols as
                # (half jh, c) for the grouped 2-octet DMA.
                yvA = y_d[:, 0:1024].rearrange(
                    "(G o v e) (jh c) -> G v e o jh c", o=2, v=4, e=2, jh=2)
                for t in range(16):
                    if t % 2 == 0:
                        ysb4 = yp.tile([128, 4, 512], f32, tag="ysb4")
                    Hts = []
                    for idx8 in range(8):
                        il = 8 * t + idx8
                        Ht = Hp.tile([128, 1024], bf16, tag="HA")
                        nc.vector.tensor_scalar(
                            Ht, rT_t[:, 0:1024],
                            aTb1_t[:, il:il + 1], 0.0, ADD, op1=MAX)
                        Hts.append(Ht)
                    py2 = pyp.tile([128, 2, 512], f32, tag="py")
                    for jh in range(2):
                        h2s_tiles = []
                        for half in range(2):
                            ph = ph2p.tile([128, 1024], f32, tag="ph")
                            for q in range(2):
                                for e in range(2):
                                    idx8 = 4 * half + 2 * q + e
                                    nc.tensor.matmul(
                                        ph[64 * e:64 * (e + 1),
                                           q * 512:(q + 1) * 512],
                                        lhsT=w2b_t,
                                        rhs=Hts[idx8][:, jh * 512:(jh + 1) * 512],
                                        start=True, stop=True,
                                        tile_position=(0, 64 * e))
                            h2s = h2p.tile([128, 1024], bf16, tag="h2s")
                            nc.scalar.activation(h2s, ph, Relu, bias=b2s_t)
                            h2s_tiles.append(h2s)
                        for v in range(4):
                            half, q = divmod(v, 2)
                            nc.tensor.matmul(
                                py2[32 * v:32 * v + 32, jh, :], lhsT=w3s_t,
                                rhs=h2s_tiles[half][:, q * 512:(q + 1) * 512],
                                start=True, stop=True,
                                tile_position=(0, 32 * v))
                    o = t % 2
                    dst = ysb4[:, 2 * o:2 * o + 2, :]
                    if o == 0:
                        nc.scalar.activation(dst, py2, Identity, bias=b3c_t)
                    else:
                        nc.vector.tensor_scalar(dst, py2, b3c_t, None, ADD)
                    if o == 1:
                        G = t // 2
                        for v in range(4):
                            nc.sync.dma_start(
                                out=yvA[G, v],
                                in_=ysb4[32 * v:32 * v + 2, :, :].rearrange(
                                    "p (o jh) c -> p o jh c", o=2))
                # Unit B: rows of a k>=4 block vs window [512,1024).
                yvB = y_d[:, 1024:1536].rearrange(
                    "(o v e) f -> v e o f", v=4, e=2)
                for t in range(16):
                    if t % 4 == 0:
                        ysb4 = yp.tile([128, 4, 512], f32, tag="ysb4")
                    if t % 2 == 0:
                        py2 = pyp.tile([128, 2, 512], f32, tag="py")
                    h2s_tiles = []
                    for half in range(2):
                        ph = ph2p.tile([128, 1024], f32, tag="ph")
                        for q in range(2):
                            for e in range(2):
                                il = 128 + 8 * t + 4 * half + 2 * q + e
                                Ht = Hp.tile([128, 512], bf16, tag="HB")
                                nc.vector.tensor_scalar(
                                    Ht, rT_t[:, 1024:1536],
                                    aTb1_t[:, il:il + 1], 0.0, ADD, op1=MAX)
                                nc.tensor.matmul(
                                    ph[64 * e:64 * (e + 1),
                                       q * 512:(q + 1) * 512],
                                    lhsT=w2b_t, rhs=Ht, start=True, stop=True,
                                    tile_position=(0, 64 * e))
                        h2s = h2p.tile([128, 1024], bf16, tag="h2s")
                        nc.scalar.activation(h2s, ph, Relu, bias=b2s_t)
                        h2s_tiles.append(h2s)
                    for v in range(4):
                        half, q = divmod(v, 2)
                        nc.tensor.matmul(
                            py2[32 * v:32 * v + 32, t % 2, :], lhsT=w3s_t,
                            rhs=h2s_tiles[half][:, q * 512:(q + 1) * 512],
                            start=True, stop=True,
                            tile_position=(0, 32 * v))
                    if t % 2 == 1:
                        dst = ysb4[:, t % 4 - 1:t % 4 + 1, :]
                        if t % 4 == 1:
                            nc.scalar.activation(dst, py2, Identity,
                                                 bias=b3c_t)
                        else:
                            nc.vector.tensor_scalar(dst, py2, b3c_t,
                                                    None, ADD)
                    if t % 4 == 3:
                        g = t // 4
                        for v in range(4):
                            nc.sync.dma_start(
                                out=yvB[v, :, 4 * g:4 * g + 4, :],
                                in_=ysb4[32 * v:32 * v + 2, :, :])
    return nc


def _pack_inputs(x, W1, b1, W2, b2, W3, b3):
    xT = np.ascontiguousarray(x.transpose(0, 2, 1)).astype(np.float32)  # [2,C,N]
    w1l = np.ascontiguousarray(W1[:C]).astype(np.float32)
    w1r = np.ascontiguousarray(W1[C:]).astype(np.float32)
    b1c = np.ascontiguousarray(b1.reshape(H1, 1)).astype(np.float32)
    w2b = np.ascontiguousarray(W2).astype(BF16)
    b2s = np.concatenate([b2, b2]).reshape(128, 1).astype(np.float32)
    w3s = np.zeros((128, 32), dtype=BF16)
    w3s[0:64, 0] = W3[:, 0].astype(BF16)
    w3s[64:128, 1] = W3[:, 0].astype(BF16)
    b3c = np.full((128, 1), b3[0], dtype=np.float32)

    in_maps = []
    for c in range(NCORES):
        (bA, kA), (bB, kB) = CORE_UNITS[c]
        xr = np.concatenate([xT[bA][:, 128 * kA:128 * kA + 128],
                             xT[bB][:, 128 * kB:128 * kB + 128]], axis=1)
        xw = np.concatenate([xT[bA][:, 0:1024], xT[bB][:, 512:1024]], axis=1)
        in_maps.append({
            "xr": np.ascontiguousarray(xr), "xw": np.ascontiguousarray(xw),
            "w1l": w1l, "w1r": w1r, "b1c": b1c, "w2b": w2b, "b2s": b2s,
            "w3s": w3s, "b3c": b3c,
        })
    return in_maps


def _assemble(results):
    global _TRIU
    y = np.zeros((B, N, N), dtype=np.float32)
    for c in range(NCORES):
        out = results[c]["y"]          # [128, 1536]
        (bA, kA), (bB, kB) = CORE_UNITS[c]
        y[bA, 128 * kA:128 * kA + 128, 0:1024] = out[:, 0:1024]
        y[bB, 128 * kB:128 * kB + 128, 512:1024] = out[:, 1024:1536]
    if _TRIU is None:
        _TRIU = np.triu(np.ones((N, N), dtype=np.float32), k=1)
    y *= _TRIU
    return y


def kernel(x, W1, b1, W2, b2, W3, b3):
    import os
    _install_compile_patch()
    from concourse.bass_utils import run_bass_kernel_spmd

    trace = bool(int(os.environ.get("FC_TRACE", "0")))
    nc = _build_program()
    in_maps = _pack_inputs(np.asarray(x), np.asarray(W1), np.asarray(b1),
                           np.asarray(W2), np.asarray(b2), np.asarray(W3),
                           np.asarray(b3))
    res = run_bass_kernel_spmd(nc, in_maps, core_ids=list(range(NCORES)),
                               trace=trace)
    LAST_PERF.clear()
    LAST_PERF.update({
        "exec_time_ns": res.exec_time_ns,
        "mean_exec_time_ns": res.mean_exec_time_ns,
        "trace": res.instructions_and_trace[1] if res.instructions_and_trace else None,
    })
    return _assemble(res.results)
